# revision 40
# baseline (speedup 1.0000x reference)
"""Trainium2 Bass kernel for conv-projected multi-head attention (v5).

Reference computation (per batch item b of 8, one NeuronCore each):
  y   = BN(depthwise3x3(x_b reshaped to [C,32,32]))      # q = k = v = y
  q/k/v = y @ w{q,k,v}^T  (heads: 12 x 32)
  att = softmax((q @ k^T) * sqrt(32))
  out = (att @ v) @ wo^T

v5 vs the fp32r baseline (263 us):
 - conv, q/k/v/out projections and the score matmuls run with bf16
   operands (psum stays f32); sqrt(32) folded into wq host-side.
   NOTE: exp with bf16 output miscompiles (writes raw f32) and walrus
   rejects mixed 32/16-bit matmul inputs, so E and vaug stay f32r and
   PV runs f32r like the baseline.
 - reciprocal_approx_fast replaces the 3.3us-per-call precise reciprocal
 - software-pipelined emission: scores+exp of block k+1 are emitted
   around PV of block k so ACT(exp) always has a backlog; attention for
   group 0 is emitted before the v projection / qk groups 1-2 so the
   scalar engine starts early.
Layout is channel-major: xT [C=384, T=1024] per core; S^T[t, l] per head;
vaug [t, h, 34] with a ones column so PV also yields the softmax
denominators in psum row 32 (gathered via ind4 matmuls, broadcast via a
K=4 bind matmul, exactly as the baseline).
"""
import sys

sys.path.insert(0, "/opt/trn_rl_repo")
from contextlib import ExitStack

import numpy as np

B, T, C = 8, 1024, 384
NH, DH = 12, 32
HH = WW = 32
SCALE = float(DH) ** 0.5
BN_EPS = 1e-5
NCORES = 8

_CACHE = {}


def _build(debug=False):
    import concourse.bass as bass
    import concourse.tile as tile
    from concourse import bacc, mybir
    from concourse.masks import make_identity

    F32 = mybir.dt.float32
    F32R = mybir.dt.float32r
    BF16 = mybir.dt.bfloat16
    AF = mybir.ActivationFunctionType
    ALU = mybir.AluOpType

    nc = bacc.Bacc("TRN2", target_bir_lowering=False, debug=False)

    xp_d = nc.dram_tensor("xp", [C, 34 * 34], BF16, kind="ExternalInput").ap()
    diag_d = nc.dram_tensor("diag", [C, 9, 128], BF16, kind="ExternalInput").ap()
    bias_d = nc.dram_tensor("bias", [C, 1], F32, kind="ExternalInput").ap()
    wqT_d = nc.dram_tensor("wqT", [C, C], BF16, kind="ExternalInput").ap()
    wkT_d = nc.dram_tensor("wkT", [C, C], BF16, kind="ExternalInput").ap()
    wvT_d = nc.dram_tensor("wvT", [C, C], BF16, kind="ExternalInput").ap()
    woT_d = nc.dram_tensor("woT", [C, C], BF16, kind="ExternalInput").ap()
    ind_d = nc.dram_tensor("ind", [4, 34, 128], F32R, kind="ExternalInput").ap()
    bind_d = nc.dram_tensor("bind", [4, 128], F32R, kind="ExternalInput").ap()
    outT_d = nc.dram_tensor("outT", [C, T], F32, kind="ExternalOutput").ap()
    dbg = {}
    if debug:
        dbg["y"] = nc.dram_tensor("dbg_y", [C, T], BF16, kind="ExternalOutput").ap()
        dbg["qT"] = nc.dram_tensor("dbg_qT", [C, T], BF16, kind="ExternalOutput").ap()
        dbg["attn"] = nc.dram_tensor(
            "dbg_attn", [C, T], BF16, kind="ExternalOutput"
        ).ap()
        dbg["E"] = nc.dram_tensor(
            "dbg_E", [8, 128, 2048], BF16, kind="ExternalOutput"
        ).ap()

    CT = C // 128  # 3 c-tiles / head groups of 4
    TT = T // 128  # 8 t-tiles
    TH = T // 512  # 2 l-halves
    DBG_BLOCK = (1, 0)  # (lh, g) block to dump in debug mode

    with tile.TileContext(nc) as tc, ExitStack() as top:
        persist = top.enter_context(tc.tile_pool(name="persist", bufs=1))
        copies = top.enter_context(tc.tile_pool(name="copies", bufs=3))

        y_sb = [persist.tile([128, T], BF16, tag=f"y{i}", name=f"y{i}") for i in range(CT)]
        qT_sb = [persist.tile([128, T], BF16, tag=f"q{i}", name=f"q{i}") for i in range(CT)]
        kT_sb = [persist.tile([128, T], BF16, tag=f"k{i}", name=f"k{i}") for i in range(CT)]
        vaug = [persist.tile([128, NH, 34], BF16, tag=f"va{i}", name=f"va{i}") for i in range(TT)]
        attn_sb = [persist.tile([128, T], BF16, tag=f"at{i}", name=f"at{i}") for i in range(CT)]

        # PSUM banks (8 x 2KB), pools strictly LIFO-nested per space:
        #   s_ps 4 (one s4 [128,2048], bufs=1) spans the whole kernel
        #   conv window:  s 4 + conv_ps 2                 = 6
        #   qk/v window:  s 4 + qk_ps 2 + v_ps 2          = 8
        #   attention:    s 4 + ov 2 + sg 1 + rb 1        = 8
        #   out-proj:     s 4 + o_ps 3                    = 7
        s_ps = top.enter_context(tc.tile_pool(name="s_ps", bufs=2, space="PSUM"))

        # ---------------- phase 1: conv ----------------
        ph_conv = ExitStack()
        convpool = ph_conv.enter_context(tc.tile_pool(name="convpool", bufs=1))
        conv_ps = ph_conv.enter_context(tc.tile_pool(name="conv_ps", bufs=2, space="PSUM"))
        ph1 = ExitStack()  # qk_ps/v_ps entered after conv pools close

        xp = [convpool.tile([128, 34 * 34], BF16, tag=f"xp{i}", name=f"xp{i}") for i in range(CT)]
        diag = [convpool.tile([128, 9, 128], BF16, tag=f"dg{i}", name=f"dg{i}") for i in range(CT)]

        for tt in range(TT):
            nc.gpsimd.memset(vaug[tt][:, :, 32:34], 0.0)
            nc.gpsimd.memset(vaug[tt][:, :, 32:33], 1.0)
        for i in range(CT):
            nc.sync.dma_start(
                xp[i][:, 0:612], xp_d[i * 128 : (i + 1) * 128, 0:612]
            )
            nc.sync.dma_start(
                diag[i][:, 0:1, :].rearrange("p a b -> p (a b)"),
                diag_d[i * 128 : (i + 1) * 128, 0:1].rearrange("c a b -> c (a b)"),
            )
            nc.sync.dma_start(
                xp[i][:, 544:1156], xp_d[i * 128 : (i + 1) * 128, 544:1156]
            )
            nc.sync.dma_start(
                diag[i][:, 1:9, :].rearrange("p a b -> p (a b)"),
                diag_d[i * 128 : (i + 1) * 128, 1:9].rearrange("c a b -> c (a b)"),
            )

        # bias + weights on the vector queue, ind/bind on gpsimd: the sync
        # queue then carries only the 6 conv-input DMAs, so conv starts early
        # and its bias-add never waits behind the weight transfers.
        bias_sb = [persist.tile([128, 1], F32, tag=f"b{i}", name=f"b{i}") for i in range(CT)]
        for i in range(CT):
            nc.scalar.dma_start(bias_sb[i][:], bias_d[i * 128 : (i + 1) * 128, :])
        wT_sb = {}
        for nm, d in (("q", wqT_d), ("k", wkT_d), ("v", wvT_d), ("o", woT_d)):
            tiles = [persist.tile([128, C], BF16, tag=f"w{nm}{i}", name=f"w{nm}{i}") for i in range(CT)]
            for i in range(CT):
                nc.scalar.dma_start(tiles[i][:], d[i * 128 : (i + 1) * 128, :])
            wT_sb[nm] = tiles
        ind4 = []
        for j in range(4):
            it = persist.tile([34, 128], F32R, tag=f"ind{j}", name=f"ind{j}")
            nc.gpsimd.dma_start(it[:], ind_d[j])
            ind4.append(it)
        bind = persist.tile([4, 128], F32R, tag="bind", name="bind")
        nc.gpsimd.dma_start(bind[:], bind_d)

        # conv: 9 accumulating diag matmuls per (c-tile, t-half)
        for i in range(CT):
            for th in range(TH):
                yp = conv_ps.tile([128, 512], F32, tag="conv", name=f"yp{i}{th}")
                r0 = th * 16
                for k in range(9):
                    dy, dx = k // 3 - 1, k % 3 - 1
                    off = (r0 + 1 + dy) * 34 + (1 + dx)
                    rhs = bass.AP(
                        tensor=xp[i].tensor,
                        offset=xp[i].offset + off,
                        ap=[list(p) for p in xp[i].ap[:1]] + [[34, 16], [1, 32]],
                    )
                    nc.tensor.matmul(
                        yp[:].rearrange("p (a b) -> p a b", a=16),
                        diag[i][:, k, :],
                        rhs,
                        start=(k == 0),
                        stop=(k == 8),
                    )
                nc.vector.tensor_scalar_add(
                    y_sb[i][:, th * 512 : (th + 1) * 512], yp[:], bias_sb[i][:]
                )
        if debug:
            for i in range(CT):
                nc.sync.dma_start(dbg["y"][i * 128 : (i + 1) * 128, :], y_sb[i][:])

        ps = {}
        pools = {}

        def qk_proj(ot):
            for nm, dst in (("q", qT_sb), ("k", kT_sb)):
                for th in range(TH):
                    pp = ps["qk"].tile([128, 512], F32, tag="qk", name=f"pp{nm}{ot}{th}")
                    for kt in range(CT):
                        nc.tensor.matmul(
                            pp[:],
                            wT_sb[nm][kt][:, ot * 128 : (ot + 1) * 128],
                            y_sb[kt][:, th * 512 : (th + 1) * 512],
                            start=(kt == 0),
                            stop=(kt == CT - 1),
                        )
                    nc.vector.tensor_copy(dst[ot][:, th * 512 : (th + 1) * 512], pp[:])

        def v_proj():
            for tt in range(TT):
                vp = ps["v"].tile([128, C], F32, tag="v", name=f"vp{tt}")
                for kt in range(CT):
                    nc.tensor.matmul(
                        vp[:],
                        y_sb[kt][:, tt * 128 : (tt + 1) * 128],
                        wT_sb["v"][kt][:],
                        start=(kt == 0),
                        stop=(kt == CT - 1),
                    )
                nc.vector.tensor_copy(
                    vaug[tt][:, :, 0:32], vp[:].rearrange("p (h d) -> p h d", h=NH)
                )

        def scores_block(lh, g):
            """Scores + exp for (l-half lh, head-group g). Returns E[p][tt]
            tiles, each [128, 1024] covering head pair (2p, 2p+1)."""
            E = [[None] * TT for _ in range(2)]
            for p in range(2):
                for tt in range(TT):
                    s2 = s_ps.tile([128, 1024], F32, tag="s2", name=f"s{lh}{g}{p}{tt}")
                    for hx in range(2):
                        hh = 2 * p + hx
                        nc.tensor.matmul(
                            s2[:, 512 * hx : 512 * (hx + 1)],
                            kT_sb[g][32 * hh : 32 * (hh + 1), tt * 128 : (tt + 1) * 128],
                            qT_sb[g][32 * hh : 32 * (hh + 1), lh * 512 : (lh + 1) * 512],
                            start=True,
                            stop=True,
                            tile_position=(32 * hh, 0),
                        )
                    e = pools["e"].tile([128, 1024], BF16, tag="E", name=f"E{lh}{g}{p}{tt}")
                    nc.scalar.activation(e[:], s2[:], AF.Exp)
                    E[p][tt] = e
            if debug and (lh, g) == DBG_BLOCK:
                for tt in range(TT):
                    nc.sync.dma_start(dbg["E"][tt][:, 0:1024], E[0][tt][:])
                    nc.sync.dma_start(dbg["E"][tt][:, 1024:2048], E[1][tt][:])
            return E

        def pv_block(lh, g, E):
            """PV + normalize for (lh, g) consuming that block's E tiles."""
            ovs_g = []
            sg = ps["sg"].tile([128, 512], F32, tag="sg", name=f"sg{lh}{g}")
            for hl in range(4):
                ov = ps["ov"].tile([128, 512], F32, tag="ov", name=f"ov{lh}{g}{hl}")
                for tt in range(TT):
                    nc.tensor.matmul(
                        ov[0:34, :],
                        vaug[tt][:, 4 * g + hl, :],
                        E[hl // 2][tt][:, 512 * (hl % 2) : 512 * (hl % 2 + 1)],
                        start=(tt == 0),
                        stop=(tt == TT - 1),
                    )
                ovs = pools["ov"].tile([128, 512], F32R, tag="ovs", name=f"ovs{lh}{g}{hl}")
                nc.vector.tensor_copy(ovs[0:34, :], ov[0:34, :])
                ovs_g.append(ovs)
                nc.tensor.matmul(
                    sg[:],
                    ind4[hl][:],
                    ovs[0:34, :],
                    start=(hl == 0),
                    stop=(hl == 3),
                )
            rrf = pools["r"].tile([128, 512], F32, tag="rrf", name=f"rrf{lh}{g}")
            nc.vector.reciprocal_approx_fast(rrf[0:4, :], sg[0:4, :])
            rr = pools["r"].tile([128, 512], F32R, tag="rr", name=f"rr{lh}{g}")
            nc.vector.tensor_copy(rr[0:4, :], rrf[0:4, :])
            Rb = ps["rb"].tile([128, 512], F32, tag="Rb", name=f"Rb{lh}{g}")
            nc.tensor.matmul(
                Rb[:], bind[:], rr[0:4, :], start=True, stop=True
            )
            for hl in range(4):
                nc.vector.tensor_tensor(
                    attn_sb[g][32 * hl : 32 * (hl + 1), lh * 512 : (lh + 1) * 512],
                    ovs_g[hl][0:32, :].bitcast(F32),
                    Rb[32 * hl : 32 * (hl + 1), :],
                    ALU.mult,
                )

        # ---------------- schedule ----------------
        ph_conv.close()
        pools["e"] = top.enter_context(tc.tile_pool(name="epool", bufs=32))
        pools["r"] = top.enter_context(tc.tile_pool(name="rpool", bufs=3))
        pools["ov"] = top.enter_context(tc.tile_pool(name="ovpool", bufs=6))
        ps["qk"] = ph1.enter_context(tc.tile_pool(name="qk_ps", bufs=2, space="PSUM"))
        ps["v"] = ph1.enter_context(tc.tile_pool(name="v_ps", bufs=2, space="PSUM"))
        qk_proj(0)
        E00 = scores_block(0, 0)
        E10 = scores_block(1, 0)
        v_proj()
        qk_proj(1)
        qk_proj(2)
        ph1.close()
        ph2 = ExitStack()
        ps["ov"] = ph2.enter_context(tc.tile_pool(name="ov_ps", bufs=2, space="PSUM"))
        ps["sg"] = ph2.enter_context(tc.tile_pool(name="sg_ps", bufs=1, space="PSUM"))
        ps["rb"] = ph2.enter_context(tc.tile_pool(name="rb_ps", bufs=1, space="PSUM"))

        pv_block(0, 0, E00)
        E01 = scores_block(0, 1)
        pv_block(1, 0, E10)
        E11 = scores_block(1, 1)
        pv_block(0, 1, E01)
        E02 = scores_block(0, 2)
        pv_block(1, 1, E11)
        E12 = scores_block(1, 2)
        pv_block(0, 2, E02)
        pv_block(1, 2, E12)
        ph2.close()
        if debug:
            for i in range(CT):
                nc.sync.dma_start(dbg["qT"][i * 128 : (i + 1) * 128, :], qT_sb[i][:])
                nc.sync.dma_start(dbg["attn"][i * 128 : (i + 1) * 128, :], attn_sb[i][:])

        # ---------------- output projection ----------------
        with tc.tile_pool(name="o_ps", bufs=3, space="PSUM") as o_ps:
            for ot in range(CT):
                for th in range(TH):
                    op = o_ps.tile([128, 512], F32, tag="o", name=f"op{ot}{th}")
                    for kt in range(CT):
                        nc.tensor.matmul(
                            op[:],
                            wT_sb["o"][kt][:, ot * 128 : (ot + 1) * 128],
                            attn_sb[kt][:, th * 512 : (th + 1) * 512],
                            start=(kt == 0),
                            stop=(kt == CT - 1),
                        )
                    oc = copies.tile([128, 512], F32, tag="oc", name=f"oc{ot}{th}")
                    nc.vector.tensor_copy(oc[:], op[:])
                    nc.sync.dma_start(
                        outT_d[ot * 128 : (ot + 1) * 128, th * 512 : (th + 1) * 512],
                        oc[:],
                    )

    nc.compile()
    return nc


def _prep_inputs(x, conv_w, bn_gamma, bn_beta, bn_mean, bn_var, wq, wk, wv, wo):
    import ml_dtypes

    f32 = np.float32
    bf16 = ml_dtypes.bfloat16
    inv = (bn_gamma / np.sqrt(bn_var + BN_EPS)).astype(f32)
    w9 = (conv_w.reshape(C, 9) * inv[:, None]).astype(f32)
    bias = (bn_beta - bn_mean * inv).astype(f32).reshape(C, 1)
    diag = np.zeros((C, 9, 128), f32)
    cc = np.arange(C)
    diag[cc[:, None], np.arange(9)[None, :], (cc % 128)[:, None]] = w9
    diag = diag.astype(bf16)
    wqT = np.ascontiguousarray((np.asarray(wq, f32) * f32(SCALE)).T).astype(bf16)
    wkT = np.ascontiguousarray(np.asarray(wk, f32).T).astype(bf16)
    wvT = np.ascontiguousarray(np.asarray(wv, f32).T).astype(bf16)
    woT = np.ascontiguousarray(np.asarray(wo, f32).T).astype(bf16)
    ind = np.zeros((4, 34, 128), f32)
    for j in range(4):
        ind[j, 32, j] = 1.0
    bind = np.zeros((4, 128), f32)
    for j in range(4):
        bind[j, 32 * j : 32 * (j + 1)] = 1.0
    maps = []
    for b in range(B):
        xt = np.ascontiguousarray(np.asarray(x[b], f32).T)
        xpad = np.zeros((C, 34, 34), f32)
        xpad[:, 1:33, 1:33] = xt.reshape(C, 32, 32)
        maps.append(
            {
                "xp": xpad.reshape(C, 34 * 34).astype(bf16),
                "diag": diag,
                "bias": bias,
                "wqT": wqT,
                "wkT": wkT,
                "wvT": wvT,
                "woT": woT,
                "ind": ind,
                "bind": bind,
            }
        )
    return maps


def kernel(x, conv_w, bn_gamma, bn_beta, bn_mean, bn_var, wq, wk, wv, wo, h, w,
           **kw):
    assert int(h) == HH and int(w) == WW
    from concourse.bass_utils import run_bass_kernel_spmd

    if "nc" not in _CACHE:
        _CACHE["nc"] = _build()
    nc = _CACHE["nc"]
    maps = _prep_inputs(
        x, conv_w, bn_gamma, bn_beta, bn_mean, bn_var, wq, wk, wv, wo
    )
    res = run_bass_kernel_spmd(nc, maps, list(range(NCORES)))
    out = np.stack([res.results[b]["outT"].T for b in range(B)])
    return out.astype(np.float32)
